# revision 4
# baseline (speedup 1.0000x reference)
"""Trainium2 Bass kernel for a 6-layer GPT forward (nn_GPT_21019569946962), v2.

Sharding: 8 cores = 2 batches x 4 ranks. Rank j owns token chunks j and 7-j
(128 tokens each) of its batch for the residual stream and MLP
(sequence-parallel), and heads 3j..3j+3 for attention (head-parallel).

Per layer: LN1 per 128-token chunk -> AllGather x within the 4-core batch
group (one per chunk, overlapped with the other chunk's MLP) -> each core
computes Q/K/V for its 3 heads over all 1024 tokens -> causal attention with
STATIC lower-triangular block skip (the (q-tile, key-block) grid is global,
identical on every core; rank only selects which head weights it was fed) ->
row-parallel WO -> chunked ReduceScatter back to token owners -> residual ->
LN2 -> MLP (token-chunked so the next layer's AllGather overlaps MLP).

LN gamma is folded into downstream weights host-side; LN beta terms are
carried via copy-stage biases / host postfix (all zero for this model). The
LM head is vocab-sharded 6288/core; output is written PSUM->DRAM directly.
"""

import sys

sys.path.insert(0, "/opt/trn_rl_repo")

import numpy as np
import ml_dtypes

import concourse.bass as bass
import concourse.tile as tile
import concourse.mybir as mybir
from concourse import bacc
from concourse import bass_utils

BF16 = mybir.dt.bfloat16
F32 = mybir.dt.float32
AF = mybir.ActivationFunctionType
ALU = mybir.AluOpType

N_CORES = 8
NL = 6
D = 768
DT = 6            # d-tiles of 128
HPC = 3           # heads per core
HD = 64
DFF = 3072
VOC = 50304
VS = VOC // N_CORES
B, L = 2, 1024
EPS = 1e-6
GROUPS = [[0, 1, 2, 3], [4, 5, 6, 7]]
ALLG = [[0, 1, 2, 3, 4, 5, 6, 7]]


class GptKernel:
    def __init__(self, reps=1):
        self.reps = reps
        self.nc = self._build()

    # -------------------------------------------------------------- build
    def _build(self):
        nc = bacc.Bacc("TRN2", target_bir_lowering=False, debug=False,
                       enable_asserts=True, num_devices=N_CORES)
        self.nc = nc

        def din(name, shape, dt):
            return nc.dram_tensor(name, shape, dt, kind="ExternalInput").ap()

        self.x0 = din("x0", [D, 256], F32)        # cols: [chunk j | chunk 7-j]
        self.wq = din("wq", [NL, D, HPC * HD], BF16)  # head-sliced, g1-folded
        self.wk = din("wk", [NL, D, HPC * HD], BF16)
        self.wv = din("wv", [NL, D, HPC * HD], BF16)
        self.wo = din("wo", [NL, HPC * HD, D], BF16)  # rows for own heads
        self.w1 = din("w1", [NL, D, DFF], BF16)       # g2-folded
        self.w2 = din("w2", [NL, DFF, D], BF16)
        self.w1b = din("w1b", [NL, 1, DFF], BF16)     # w1_b + b2@w1
        self.w2b = din("w2b", [NL, 1, D], BF16)
        self.qb = din("qb", [NL, 128, 2], F32)        # b1 @ wq slice, padded
        self.wob = din("wob", [NL, 128, DT], F32)     # (b1@wv@wo)/4
        self.headw = din("headw", [D, VS], BF16)      # gf-folded slice
        self.tri = din("tri", [128, 128], F32)        # causal in-block mask
        self.out = nc.dram_tensor("out", [16 * 128, VS], F32,
                                  kind="ExternalOutput").ap()

        with tile.TileContext(nc) as tc:
            self.tc = tc
            with (
                tc.tile_pool(name="const", bufs=1) as cp,
                tc.tile_pool(name="persist", bufs=1) as pp,
                tc.tile_pool(name="psum", bufs=1, space="PSUM") as psum,
                tc.tile_pool(name="dram", bufs=1, space="DRAM") as dram,
                tc.tile_pool(name="work", bufs=1) as wp,
            ):
                self.psum, self.dram, self.wp = psum, dram, wp
                self.ones_r = cp.tile([1, 128], F32)
                nc.vector.memset(self.ones_r[:], 1.0)
                self.ones_c = cp.tile([128, 1], BF16)
                nc.vector.memset(self.ones_c[:], 1.0)
                self.ones_rb = cp.tile([1, 128], BF16)
                nc.vector.memset(self.ones_rb[:], 1.0)
                self.eps1 = cp.tile([1, 1], F32)
                nc.vector.memset(self.eps1[:], EPS)
                self.tri_sb = cp.tile([128, 128], F32)
                nc.sync.dma_start(self.tri_sb[:], self.tri)
                self.xres = pp.tile([128, DT, 256], F32)

                for rep in range(self.reps):
                    nm0 = f"r{rep}"
                    nc.sync.dma_start(
                        self.xres[:],
                        self.x0.rearrange("(dt p) t -> p dt t", p=128))
                    gos = []
                    for ch in range(2):
                        ln = self._layernorm_chunk(self.xres, ch,
                                                   f"{nm0}l0c{ch}")
                        gos.append(self._issue_gather(ln, f"{nm0}p{ch}"))
                    for l in range(NL):
                        gos = self._layer(l, rep, gos)
                    self._lm_head(rep, gos)
        nc.compile()
        return nc

    # -------------------------------------------------- layernorm on chunk
    def _layernorm_chunk(self, xres, ch, name):
        """LN over features for 128 tokens; gamma/beta folded downstream.
        Returns (x - mu) * rstd as bf16 [128, DT, 128]."""
        nc, wp, psum = self.nc, self.wp, self.psum
        cols = slice(ch * 128, ch * 128 + 128)
        xb = wp.tile([128, DT, 128], BF16, tag="xb", bufs=2, name=f"xb_{name}")
        nc.vector.tensor_copy(xb[:], xres[:, :, cols])
        xq = wp.tile([128, DT, 128], BF16, tag="xq", bufs=1, name=f"xq_{name}")
        nc.vector.tensor_mul(xq[:], xb[:], xb[:])
        stat_s = psum.tile([1, 128], F32, tag="small", bufs=2, name=f"ss_{name}")
        stat_q = psum.tile([1, 128], F32, tag="small", bufs=2, name=f"sq_{name}")
        for k in range(DT):
            nc.tensor.matmul(stat_s[:], self.ones_c[:], xb[:, k, :],
                             start=(k == 0), stop=(k == DT - 1))
        for k in range(DT):
            nc.tensor.matmul(stat_q[:], self.ones_c[:], xq[:, k, :],
                             start=(k == 0), stop=(k == DT - 1))
        mu = wp.tile([1, 128], F32, tag="lnsc", bufs=6, name=f"mu_{name}")
        nc.vector.tensor_scalar_mul(mu[:], stat_s[:], 1.0 / D)
        msq = wp.tile([1, 128], F32, tag="lnsc", bufs=6, name=f"msq_{name}")
        nc.vector.tensor_scalar_mul(msq[:], stat_q[:], 1.0 / D)
        mu2 = wp.tile([1, 128], F32, tag="lnsc", bufs=6, name=f"mu2_{name}")
        nc.vector.tensor_mul(mu2[:], mu[:], mu[:])
        vr = wp.tile([1, 128], F32, tag="lnsc", bufs=6, name=f"vr_{name}")
        nc.vector.tensor_sub(vr[:], msq[:], mu2[:])
        sd = wp.tile([1, 128], F32, tag="lnsc", bufs=6, name=f"sd_{name}")
        nc.scalar.activation(sd[:], vr[:], AF.Sqrt,
                             bias=self.eps1[:], scale=1.0)
        rstd = wp.tile([1, 128], F32, tag="lnsc", bufs=6, name=f"rstd_{name}")
        nc.vector.reciprocal(rstd[:], sd[:])
        mr = wp.tile([1, 128], F32, tag="lnsc", bufs=6, name=f"mr_{name}")
        nc.vector.tensor_mul(mr[:], mu[:], rstd[:])
        bc_r = psum.tile([128, 128], F32, tag="bc", bufs=2, name=f"bcr_{name}")
        nc.tensor.matmul(bc_r[:], self.ones_r[:], rstd[:], start=True, stop=True)
        bc_m = psum.tile([128, 128], F32, tag="bc", bufs=2, name=f"bcm_{name}")
        nc.tensor.matmul(bc_m[:], self.ones_r[:], mr[:], start=True, stop=True)
        ln = wp.tile([128, DT, 128], BF16, tag="ln", bufs=3, name=f"ln_{name}")
        for k in range(DT):
            v = wp.tile([128, 128], F32, tag="lnv", bufs=2, name=f"v{k}_{name}")
            nc.vector.tensor_mul(v[:], xres[:, k, cols], bc_r[:])
            nc.vector.tensor_sub(ln[:, k, :], v[:], bc_m[:])
        return ln

    # --------------------------------------------- AllGather of ln1 chunk
    def _issue_gather(self, ln, name):
        nc, dram = self.nc, self.dram
        agin = dram.tile([128, D], BF16, tag="agin", bufs=4, name=f"agi_{name}")
        nc.sync.dma_start(agin[:], ln[:])
        agout = dram.tile([4, 128, D], BF16, tag="agout", bufs=4,
                          name=f"ago_{name}")
        nc.gpsimd.collective_compute(
            "AllGather", ALU.bypass, ins=[agin.opt()], outs=[agout.opt()],
            replica_groups=GROUPS)
        return agout

    def _issue_final(self, lnf, name):
        nc, dram = self.nc, self.dram
        fin = dram.tile([128, D], BF16, tag="agin", bufs=4, name=f"fin_{name}")
        nc.sync.dma_start(fin[:], lnf[:])
        fout = dram.tile([8, 128, D], BF16, tag="fout", bufs=2,
                         addr_space="Shared", name=f"fo_{name}")
        nc.gpsimd.collective_compute(
            "AllGather", ALU.bypass, ins=[fin.opt()], outs=[fout.opt()],
            replica_groups=ALLG)
        return fout

    # -------------------------------------------------- per-half pieces
    def _qkv_attn_half(self, half, ago, tiles, weights, nm):
        nc, wp, psum = self.nc, self.wp, self.psum
        xg, q128, q64, k128, k64, vg, aoP, ao64 = tiles
        wqt, wkt, wvt, qb = weights
        for r in range(4):
            g = r if half == 0 else 7 - r
            nc.sync.dma_start(
                xg[:, :, g * 128:(g + 1) * 128],
                ago[r].rearrange("p (k t) -> p k t", k=DT))
        tok = slice(half * 512, half * 512 + 512)
        kb0 = half * 4

        # K, Q projections (packed 2-head tile + 1-head tile)
        projs = [
            ("k1", k128, wkt, 0, 128, 0.0),
            ("k2", k64, wkt, 128, 64, 0.0),
            ("q1", q128, wqt, 0, 128, qb[0:128, 0:1]),
            ("q2", q64, wqt, 128, 64, qb[0:64, 1:2]),
        ]
        for pname, dst, wsrc, c0, npart, bias in projs:
            ps = psum.tile([128, 4, 128], F32, tag="dense", bufs=2,
                           name=f"pp_{pname}_{half}_{nm}")
            for kk in range(DT):
                nc.tensor.matmul(ps[0:npart], wsrc[:, kk, c0:c0 + npart],
                                 xg[:, kk, tok],
                                 start=(kk == 0), stop=(kk == DT - 1))
            nc.scalar.activation(dst[:, kb0:kb0 + 4, :], ps[0:npart],
                                 AF.Identity, bias=bias, scale=1.0)

        # V projection (token-major)
        for kb in range(kb0, kb0 + 4):
            pv = psum.tile([128, HPC * HD], F32, tag="dense", bufs=2,
                           name=f"pv{kb}_{nm}")
            for kk in range(DT):
                nc.tensor.matmul(pv[:], xg[:, kk, kb * 128:(kb + 1) * 128],
                                 wvt[:, kk, :],
                                 start=(kk == 0), stop=(kk == DT - 1))
            for h in range(HPC):
                nc.vector.tensor_copy(vg[:, kb, h * 65:h * 65 + 64],
                                      pv[:, h * 64:(h + 1) * 64])

        # attention for this half's q-tiles
        for qt in range(kb0, kb0 + 4):
            for h in range(HPC):
                if h < 2:
                    kst, qst, po = k128, q128, h * 64
                    dsttile, dpo = aoP, h * 64
                else:
                    kst, qst, po = k64, q64, 0
                    dsttile, dpo = ao64, 0
                nblk = qt + 1
                sg = []
                for base in range(0, nblk, 4):
                    n = min(4, nblk - base)
                    t = psum.tile([128, 4, 128], F32, tag="sc", bufs=2,
                                  name=f"sg{qt}_{h}_{base}_{nm}")
                    for i in range(n):
                        nc.tensor.matmul(
                            t[:, i, :], kst[po:po + 64, base + i, :],
                            qst[po:po + 64, qt, :], start=True, stop=True)
                    sg.append((t, base, n))
                oaug = psum.tile([65, 128], F32, tag="small", bufs=2,
                                 name=f"oa{qt}_{h}_{nm}")
                first = True
                for (t, base, n) in sg:
                    nfull = n - 1 if base + n == nblk else n
                    if nfull > 0:
                        p = wp.tile([128, 4, 128], BF16, tag="pexp",
                                    bufs=3, name=f"pe{qt}_{h}_{base}_{nm}")
                        nc.scalar.activation(p[:, 0:nfull, :],
                                             t[:, 0:nfull, :], AF.Exp,
                                             bias=0.0, scale=0.125)
                        for i in range(nfull):
                            kb = base + i
                            nc.tensor.matmul(
                                oaug[:], vg[:, kb, h * 65:h * 65 + 65],
                                p[:, i, :], start=first, stop=False)
                            first = False
                tlast, base, n = sg[-1]
                sm = wp.tile([128, 128], F32, tag="sm", bufs=2,
                             name=f"sm{qt}_{h}_{nm}")
                nc.vector.tensor_add(sm[:], tlast[:, n - 1, :], self.tri_sb[:])
                pd = wp.tile([128, 128], BF16, tag="pd", bufs=1,
                             name=f"pd{qt}_{h}_{nm}")
                nc.scalar.activation(pd[:], sm[:], AF.Exp, bias=0.0,
                                     scale=0.125)
                nc.tensor.matmul(oaug[:], vg[:, qt, h * 65:h * 65 + 65],
                                 pd[:], start=first, stop=True)
                dnr = wp.tile([1, 128], F32, tag="dnr", bufs=2,
                              name=f"dn{qt}_{h}_{nm}")
                nc.vector.reciprocal(dnr[:], oaug[64:65, :])
                ou = wp.tile([64, 128], F32, tag="ou", bufs=2,
                             name=f"ou{qt}_{h}_{nm}")
                nc.scalar.activation(ou[:], oaug[0:64, :], AF.Identity,
                                     bias=0.0, scale=1.0)
                bcd = psum.tile([64, 128], F32, tag="bc", bufs=2,
                                name=f"bc{qt}_{h}_{nm}")
                nc.tensor.matmul(bcd[:], self.ones_r[:, 0:64], dnr[:],
                                 start=True, stop=True)
                nc.vector.tensor_mul(
                    dsttile[dpo:dpo + 64, qt * 128:(qt + 1) * 128],
                    ou[:], bcd[:])

    def _wo_rs(self, tg, tiles, wot128, wot64, wob, nm):
        """WO partials for token group tg (512 tokens) + chunked RS issue."""
        nc, wp, psum, dram = self.nc, self.wp, self.psum, self.dram
        _, _, _, _, _, _, aoP, ao64 = tiles
        rsin = dram.tile([4, 128, DT, 128], BF16, tag="rsin", bufs=4,
                         name=f"rsi{tg}_{nm}")
        wosb = wp.tile([128, DT, 512], BF16, tag="h1", bufs=3,
                       name=f"wos{tg}_{nm}")
        for m in range(DT):
            pwo = psum.tile([128, 512], F32, tag="dense", bufs=2,
                            name=f"pwo{tg}_{m}_{nm}")
            nc.tensor.matmul(pwo[:], wot128[:, m * 128:(m + 1) * 128],
                             aoP[:, tg * 512:(tg + 1) * 512],
                             start=True, stop=False)
            nc.tensor.matmul(pwo[:], wot64[:, m * 128:(m + 1) * 128],
                             ao64[:, tg * 512:(tg + 1) * 512],
                             start=False, stop=True)
            nc.scalar.activation(wosb[:, m, :], pwo[:], AF.Identity,
                                 bias=wob[:, m:m + 1], scale=1.0)
        for i in range(4):
            qt = tg * 4 + i
            r = qt if qt < 4 else 7 - qt
            nc.sync.dma_start(rsin[r], wosb[:, :, i * 128:(i + 1) * 128])
        rsout = dram.tile([128, DT, 128], BF16, tag="rsout", bufs=4,
                          name=f"rso{tg}_{nm}")
        nc.gpsimd.collective_compute(
            "ReduceScatter", ALU.add, ins=[rsin.opt()], outs=[rsout.opt()],
            replica_groups=GROUPS)
        return rsout

    def _mlp_chunk(self, ch, rsout, w1t, w2t, w1b, w2b, l, nm):
        """residual += RS result; LN2; MLP; residual; next LN1 + gather."""
        nc, wp, psum = self.nc, self.wp, self.psum
        cols = slice(ch * 128, ch * 128 + 128)
        rsg = wp.tile([128, DT, 128], BF16, tag="rsg", bufs=1,
                      name=f"rsg{ch}_{nm}")
        nc.sync.dma_start(rsg[:], rsout[:])
        nc.vector.tensor_add(self.xres[:, :, cols], self.xres[:, :, cols],
                             rsg[:])
        ln2 = self._layernorm_chunk(self.xres, ch, f"ln2c{ch}_{nm}")
        h1 = wp.tile([128, 24, 128], BF16, tag="h1", bufs=3,
                     name=f"h1c{ch}_{nm}")
        for mg in range(6):
            ph = psum.tile([128, 4, 128], F32, tag="dense", bufs=2,
                           name=f"ph{ch}_{mg}_{nm}")
            for mi in range(4):
                m = mg * 4 + mi
                hf, mh = (m * 128) // 1536, (m * 128) % 1536
                for kk in range(DT):
                    nc.tensor.matmul(
                        ph[:, mi, :],
                        w1t[hf][:, kk, mh:mh + 128],
                        ln2[:, kk, :], start=(kk == 0), stop=False)
                nc.tensor.matmul(ph[:, mi, :], w1b[:, m * 128:(m + 1) * 128],
                                 self.ones_rb[:], start=False, stop=True)
            nc.scalar.activation(h1[:, mg * 4:(mg + 1) * 4, :], ph[:],
                                 AF.Gelu_apprx_tanh, bias=0.0, scale=1.0)
        for mg in range(2):
            nmt = 4 if mg == 0 else 2
            pw = psum.tile([128, 4, 128], F32, tag="dense", bufs=2,
                           name=f"pw{ch}_{mg}_{nm}")
            for mi in range(nmt):
                m = mg * 4 + mi
                for kk in range(24):
                    hf, kh = kk // 12, kk % 12
                    nc.tensor.matmul(
                        pw[:, mi, :],
                        w2t[hf][:, kh, m * 128:(m + 1) * 128],
                        h1[:, kk, :], start=(kk == 0), stop=False)
                nc.tensor.matmul(pw[:, mi, :], w2b[:, m * 128:(m + 1) * 128],
                                 self.ones_rb[:], start=False, stop=True)
            nc.vector.tensor_add(
                self.xres[:, mg * 4:mg * 4 + nmt, cols],
                self.xres[:, mg * 4:mg * 4 + nmt, cols],
                pw[:, 0:nmt, :])
        if l < NL - 1:
            ln1n = self._layernorm_chunk(self.xres, ch, f"ln1c{ch}_{nm}n")
            return self._issue_gather(ln1n, f"{nm}c{ch}n")
        else:
            lnf = self._layernorm_chunk(self.xres, ch, f"lnfc{ch}_{nm}")
            return self._issue_final(lnf, f"{nm}c{ch}f")

    # ------------------------------------------------------------- layer
    def _layer(self, l, rep, gos):
        nc, wp = self.nc, self.wp
        nm = f"r{rep}l{l}"

        wqt = wp.tile([128, DT, HPC * HD], BF16, tag="wq", bufs=2,
                      name=f"wq_{nm}")
        nc.sync.dma_start(wqt[:], self.wq[l].rearrange("(k p) d -> p k d", p=128))
        wkt = wp.tile([128, DT, HPC * HD], BF16, tag="wk", bufs=2,
                      name=f"wk_{nm}")
        nc.sync.dma_start(wkt[:], self.wk[l].rearrange("(k p) d -> p k d", p=128))
        wvt = wp.tile([128, DT, HPC * HD], BF16, tag="wv", bufs=2,
                      name=f"wv_{nm}")
        nc.sync.dma_start(wvt[:], self.wv[l].rearrange("(k p) d -> p k d", p=128))
        qb = wp.tile([128, 2], F32, tag="qb", bufs=2, name=f"qb_{nm}")
        nc.sync.dma_start(qb[:], self.qb[l])

        xg = wp.tile([128, DT, 1024], BF16, tag="xg", bufs=1, name=f"xg_{nm}")
        q128 = wp.tile([128, 8, 128], BF16, tag="q128", bufs=1, name=f"q1_{nm}")
        q64 = wp.tile([64, 8, 128], BF16, tag="q64", bufs=1, name=f"q2_{nm}")
        k128 = wp.tile([128, 8, 128], BF16, tag="k128", bufs=1, name=f"k1_{nm}")
        k64 = wp.tile([64, 8, 128], BF16, tag="k64", bufs=1, name=f"k2_{nm}")
        vg = wp.tile([128, 8, HPC * 65], BF16, tag="vg", bufs=1,
                     name=f"vg_{nm}")
        nc.vector.memset(vg[:], 1.0)
        aoP = wp.tile([128, 1024], BF16, tag="aoP", bufs=1, name=f"aoP_{nm}")
        ao64 = wp.tile([64, 1024], BF16, tag="ao64", bufs=1, name=f"ao64_{nm}")
        tiles = (xg, q128, q64, k128, k64, vg, aoP, ao64)
        weights = (wqt, wkt, wvt, qb)

        self._qkv_attn_half(0, gos[0], tiles, weights, nm)
        wot128 = wp.tile([128, D], BF16, tag="wo128", bufs=2, name=f"woA_{nm}")
        nc.sync.dma_start(wot128[:], self.wo[l, 0:128, :])
        wot64 = wp.tile([64, D], BF16, tag="wo64", bufs=2, name=f"woB_{nm}")
        nc.sync.dma_start(wot64[:], self.wo[l, 128:192, :])
        wob = wp.tile([128, DT], F32, tag="wob", bufs=2, name=f"wob_{nm}")
        nc.sync.dma_start(wob[:], self.wob[l])
        w1t = []
        for half in range(2):
            t = wp.tile([128, DT, 1536], BF16, tag="w1h", bufs=2,
                        name=f"w1_{half}_{nm}")
            nc.sync.dma_start(
                t[:], self.w1[l, :, half * 1536:(half + 1) * 1536]
                .rearrange("(k p) d -> p k d", p=128))
            w1t.append(t)
        w1b = wp.tile([1, DFF], BF16, tag="w1b", bufs=2, name=f"w1b_{nm}")
        nc.sync.dma_start(w1b[:], self.w1b[l])
        rsout0 = self._wo_rs(0, tiles, wot128, wot64, wob, nm)
        self._qkv_attn_half(1, gos[1], tiles, weights, nm)
        w2t = []
        for half in range(2):
            t = wp.tile([128, 12, D], BF16, tag="w2h", bufs=2,
                        name=f"w2_{half}_{nm}")
            nc.sync.dma_start(
                t[:], self.w2[l, half * 1536:(half + 1) * 1536, :]
                .rearrange("(k p) d -> p k d", p=128))
            w2t.append(t)
        w2b = wp.tile([1, D], BF16, tag="w2b", bufs=2, name=f"w2b_{nm}")
        nc.sync.dma_start(w2b[:], self.w2b[l])
        rsout1 = self._wo_rs(1, tiles, wot128, wot64, wob, nm)
        g0 = self._mlp_chunk(0, rsout0, w1t, w2t, w1b, w2b, l, nm)
        g1 = self._mlp_chunk(1, rsout1, w1t, w2t, w1b, w2b, l, nm)
        return [g0, g1]

    # ------------------------------------------------------------ lm head
    def _lm_head(self, rep, gos):
        nc, wp, psum = self.nc, self.wp, self.psum
        nm = f"r{rep}f"
        fg = wp.tile([128, 16, DT, 128], BF16, tag="fg", bufs=1,
                     name=f"fg_{nm}")
        hts0 = wp.tile([128, DT, 1536], BF16, tag="w1h", bufs=2,
                       name=f"hw0_{nm}")
        nc.sync.dma_start(hts0[:],
                          self.headw[:, 0:1536]
                          .rearrange("(k p) d -> p k d", p=128))
        for ch in range(2):
            for r in range(8):
                nc.sync.dma_start(
                    fg[:, r * 2 + ch],
                    gos[ch][r].rearrange("p (k t) -> p k t", k=DT))

        vblocks = [(i * 1536, 1536) for i in range(4)] + [(6144, 144)]
        torder = list(range(0, 16, 2)) + list(range(1, 16, 2))
        for vb0, vbn in vblocks:
            if vb0 == 0:
                hts = hts0
            else:
                hts = wp.tile([128, DT, 1536], BF16, tag="w1h", bufs=2,
                              name=f"hw{vb0}_{nm}")
                nc.sync.dma_start(
                    hts[:, :, 0:vbn],
                    self.headw[:, vb0:vb0 + vbn]
                    .rearrange("(k p) d -> p k d", p=128))
            nvc = (vbn + 511) // 512
            for t in torder:
                for vc in range(nvc):
                    n = min(512, vbn - vc * 512)
                    ps = psum.tile([128, 512], F32, tag="dense", bufs=2,
                                   name=f"po{vb0}_{t}_{vc}_{nm}")
                    for kk in range(DT):
                        nc.tensor.matmul(
                            ps[:, 0:n], fg[:, t, kk, :],
                            hts[:, kk, vc * 512:vc * 512 + n],
                            start=(kk == 0), stop=(kk == DT - 1))
                    ot = wp.tile([128, 512], F32, tag="ot", bufs=2,
                                 name=f"ot{vb0}_{t}_{vc}_{nm}")
                    if (t + vc) % 2 == 0:
                        nc.scalar.activation(ot[:, 0:n], ps[:, 0:n],
                                             AF.Identity, bias=0.0, scale=1.0)
                    else:
                        nc.vector.tensor_copy(ot[:, 0:n], ps[:, 0:n])
                    nc.sync.dma_start(
                        self.out[t * 128:(t + 1) * 128,
                                 vb0 + vc * 512:vb0 + vc * 512 + n],
                        ot[:, 0:n])


# ----------------------------------------------------------------- host side

_CACHE = {}


def _prep_inputs(inputs):
    ids = np.asarray(inputs["input_ids"])
    tok_emb = np.asarray(inputs["tok_emb"], dtype=np.float32)
    pos_emb = np.asarray(inputs["pos_emb"], dtype=np.float32)
    x = tok_emb[ids] + pos_emb[:L][None]          # [2, 1024, 768] f32

    f32 = lambda a: np.ascontiguousarray(np.asarray(a, np.float32))
    bf = lambda a: np.ascontiguousarray(np.asarray(a, np.float32)).astype(
        ml_dtypes.bfloat16)

    g1 = f32(inputs["ln1_s"]); b1 = f32(inputs["ln1_b"])
    g2 = f32(inputs["ln2_s"]); b2 = f32(inputs["ln2_b"])
    gf = f32(inputs["lnf_s"]); bf_ = f32(inputs["lnf_b"])
    wq = f32(inputs["wq"]) * g1[:, :, None]
    wk = f32(inputs["wk"]) * g1[:, :, None]
    wv = f32(inputs["wv"]) * g1[:, :, None]
    wo = f32(inputs["wo"])
    w1 = f32(inputs["w1_k"]) * g2[:, :, None]
    w2 = f32(inputs["w2_k"])
    head = f32(inputs["head"]) * gf[:, None]

    # beta terms
    qb_full = np.einsum("ld,ldm->lm", b1, f32(inputs["wq"]))   # [NL, 768]
    vb_full = np.einsum("ld,ldm->lm", b1, f32(inputs["wv"]))   # [NL, 768]
    w1b_eff = f32(inputs["w1_b"]) + np.einsum("ld,ldm->lm", b2, f32(inputs["w1_k"]))
    headb = bf_ @ f32(inputs["head"])                          # [VOC]

    tri = np.where(np.arange(128)[:, None] <= np.arange(128)[None, :],
                   0.0, -30000.0).astype(np.float32)

    in_maps = []
    for c in range(N_CORES):
        b, j = c // 4, c % 4
        hs = slice(192 * j, 192 * (j + 1))
        m = {
            "wq": bf(wq[:, :, hs]), "wk": bf(wk[:, :, hs]),
            "wv": bf(wv[:, :, hs]), "wo": bf(wo[:, hs, :]),
            "w1": bf(w1), "w2": bf(w2),
            "w1b": bf(w1b_eff)[:, None, :], "w2b": bf(inputs["w2_b"])[:, None, :],
            "headw": bf(head[:, c * VS:(c + 1) * VS]),
            "tri": tri,
        }
        qb = np.zeros((NL, 128, 2), np.float32)
        qb[:, :, 0] = qb_full[:, hs][:, 0:128]
        qb[:, 0:64, 1] = qb_full[:, hs][:, 128:192]
        m["qb"] = qb
        wob_c = np.einsum("lm,lmd->ld", vb_full[:, hs.start:hs.stop]
                          if False else vb_full[:, hs], wo[:, hs, :])
        wob = np.zeros((NL, 128, DT), np.float32)
        for k in range(DT):
            wob[:, :, k] = wob_c[:, k * 128:(k + 1) * 128]
        m["wob"] = wob
        xa = x[b, 128 * j:128 * j + 128]           # chunk j  [128, 768]
        xb_ = x[b, 128 * (7 - j):128 * (7 - j) + 128]
        m["x0"] = np.ascontiguousarray(np.concatenate([xa, xb_], 0).T)
        in_maps.append(m)
    _CACHE["headb"] = headb
    return in_maps


def _assemble(results):
    final = np.empty((B, L, VOC), np.float32)
    for c in range(N_CORES):
        o = results[c]["out"]                     # [2048, VS]
        for r in range(8):
            bb, jj = r // 4, r % 4
            for ch in range(2):
                t = r * 2 + ch
                chunk = jj if ch == 0 else 7 - jj
                final[bb, 128 * chunk:128 * chunk + 128,
                      c * VS:(c + 1) * VS] = o[128 * t:128 * (t + 1)]
    headb = _CACHE.get("headb")
    if headb is not None and np.any(headb):
        final += headb[None, None, :]
    return final


class SpmdRunner:
    """Persistent PJRT runner: builds the shard_map jit once, keeps inputs
    staged on device, so repeated runs avoid retracing (which also breaks
    the collective mesh when done twice for one module)."""

    def __init__(self, nc, n_cores):
        import jax
        from jax.sharding import Mesh, PartitionSpec
        from jax.experimental.shard_map import shard_map
        from concourse.bass2jax import (
            _bass_exec_p, install_neuronx_cc_hook, partition_id_tensor)

        install_neuronx_cc_hook()
        self.jax = jax
        self.n_cores = n_cores
        partition_name = (nc.partition_id_tensor.name
                          if nc.partition_id_tensor else None)
        in_names, out_names, out_avals, zero_outs = [], [], [], []
        for alloc in nc.m.functions[0].allocations:
            if not isinstance(alloc, mybir.MemoryLocationSet):
                continue
            name = alloc.memorylocations[0].name
            if alloc.kind == "ExternalInput":
                if name != partition_name:
                    in_names.append(name)
            elif alloc.kind == "ExternalOutput":
                shape = tuple(alloc.tensor_shape)
                dtype = mybir.dt.np(alloc.dtype)
                out_names.append(name)
                out_avals.append(jax.core.ShapedArray(shape, dtype))
                zero_outs.append(np.zeros(shape, dtype))
        self.in_names, self.out_names = in_names, out_names
        self.out_avals = out_avals
        n_params = len(in_names)
        all_in = list(in_names) + list(out_names)
        if partition_name is not None:
            all_in.append(partition_name)
        self.dbg_extra = {}
        if nc.dbg_addr is not None:
            self.dbg_extra[nc.dbg_addr.name] = np.zeros((1, 2), np.uint32)

        def _body(*args):
            operands = list(args)
            if partition_name is not None:
                operands.append(partition_id_tensor())
            outs = _bass_exec_p.bind(
                *operands, out_avals=tuple(out_avals),
                in_names=tuple(all_in), out_names=tuple(out_names),
                lowering_input_output_aliases=(),
                sim_require_finite=True, sim_require_nnan=True, nc=nc)
            return tuple(outs)

        devices = jax.devices()[:n_cores]
        mesh = Mesh(np.asarray(devices), ("core",))
        in_specs = (PartitionSpec("core"),) * (n_params + len(out_names))
        out_specs = (PartitionSpec("core"),) * len(out_names)
        self.fn = jax.jit(
            shard_map(_body, mesh=mesh, in_specs=in_specs,
                      out_specs=out_specs, check_rep=False),
            keep_unused=True)
        self.zero_outs = zero_outs
        self.staged = None

    def stage(self, in_maps):
        n = self.n_cores
        in_maps = [dict(m, **self.dbg_extra) for m in in_maps]
        concat = [
            np.concatenate([np.asarray(in_maps[c][name]) for c in range(n)], 0)
            for name in self.in_names]
        zeros = [np.zeros((n * a.shape[0], *a.shape[1:]), a.dtype)
                 for a in self.out_avals]
        self.staged = [self.jax.device_put(x) for x in (concat + zeros)]
        self.jax.block_until_ready(self.staged)

    def run(self):
        out = self.fn(*self.staged)
        self.jax.block_until_ready(out)
        self.last_out = out
        return out

    def results(self):
        n = self.n_cores
        return [
            {name: np.asarray(self.last_out[i]).reshape(
                n, *self.out_avals[i].shape)[c]
             for i, name in enumerate(self.out_names)}
            for c in range(n)]


def kernel(**inputs):
    if "k" not in _CACHE:
        _CACHE["k"] = GptKernel(reps=1)
        _CACHE["runner"] = SpmdRunner(_CACHE["k"].nc, N_CORES)
    r = _CACHE["runner"]
    in_maps = _prep_inputs(inputs)
    r.stage(in_maps)
    r.run()
    return _assemble(r.results())
